# revision 46
# baseline (speedup 1.0000x reference)
"""DoReFa dense layer (bitW=1, bitA=3) on 8 Trainium2 NeuronCores.

out = quantize_act(clip(|x|,0,1), 3b) @ (sign(W) * mean|W|) + b

Math used by the kernel (exact):
    a_int = round(min(7*|x|, 7))   in {0..7}   -> exact in bf16/fp8
    S     = sign(W)                in {-1,0,1} -> exact in fp8
    out   = (E/7) * (a_int @ S) + b,  E = mean|W| (computed on device)

The integer matmul accumulates exactly in fp32 PSUM (|sums| <= 28672 < 2^15),
so intermediate results are stored as int16 and scaled by E/7 at the end.

Sharding: data-parallel over batch (8 x 1024 rows), W replicated.
"""

import sys

sys.path.insert(0, "/opt/trn_rl_repo")

from contextlib import ExitStack

import numpy as np
from concourse import bacc, mybir, tile
from concourse import bass_isa
from concourse.bass_utils import run_bass_kernel_spmd

# Problem dims (hardcoded per contract)
BATCH, IN_CH, N_UNITS = 8192, 4096, 4096
N_CORES = 8
P = 128

M = BATCH // N_CORES  # 1024 rows per core
KO = IN_CH // P  # 32 k-subtiles of 128
MT = M // P  # 8 m-subtiles of 128
NBS = 512  # n-block width
NB = N_UNITS // NBS  # 8 n-blocks
KC = 4  # k-subtiles per W dma chunk
NCH = KO // KC  # 8 chunks per n-block
KH = 1024  # k columns per activation quant chunk

MAGIC = float(2**23)

F32 = mybir.dt.float32
BF16 = mybir.dt.bfloat16
FP8 = mybir.dt.float8e4
I16 = mybir.dt.int16
AF = mybir.ActivationFunctionType
ALU = mybir.AluOpType


def _body(ctx, tc, x, w, b, out, use_dr, add_bias):
    nc = tc.nc

    xr = x.rearrange("(mt p) k -> mt p k", p=P)
    # row = kc*256 + 2p + t: partition p holds the adjacent row pair
    # (2p, 2p+1) of each 256-row group kc -- matches the aT u16 pairing.
    wr = w.rearrange("(kc p two) n -> p kc two n", p=P, two=2)
    outr = out.rearrange("(mt p) n -> mt p n", p=P)

    const = ctx.enter_context(tc.tile_pool(name="const", bufs=1))
    xs_pool = ctx.enter_context(tc.tile_pool(name="xs", bufs=3))
    qb_pool = ctx.enter_context(tc.tile_pool(name="qb", bufs=2))
    stg_pool = ctx.enter_context(tc.tile_pool(name="stg", bufs=4))
    ws_pool = ctx.enter_context(tc.tile_pool(name="ws", bufs=4))
    ss_pool = ctx.enter_context(tc.tile_pool(name="ss", bufs=9))
    abs_pool = ctx.enter_context(tc.tile_pool(name="abss", bufs=2))
    orow_pool = ctx.enter_context(tc.tile_pool(name="orow", bufs=3))
    psum_pool = ctx.enter_context(tc.tile_pool(name="psum", bufs=8, space="PSUM"))
    dram_pool = ctx.enter_context(tc.tile_pool(name="dram", bufs=1, space="DRAM"))

    # Resident tensors (all fp8 activations: 32KB/partition)
    if use_dr:
        # ko-pair tiles for DoubleRow: [p, 2, M] fp8
        aT = [const.tile([P, 2, M], FP8, name=f"aT{i}") for i in range(KO // 2)]
    else:
        aT = [const.tile([P, M], FP8, name=f"aT{i}") for i in range(KO)]
    unscaled = [const.tile([P, N_UNITS], I16, name=f"uns{m}") for m in range(MT)]
    accW = const.tile([P, NB * NCH], F32, name="accW")
    sAP = const.tile([P, 1], F32, name="sAP")

    if add_bias:
        b_bc = const.tile([P, N_UNITS], F32, name="b_bc")
        nc.scalar.dma_start(b_bc[0:1, :], b[:])
        nc.gpsimd.partition_broadcast(b_bc[:], b_bc[0:1, :], channels=P)

    # ---- Phase A + B interleaved ----
    # Phase A: a_q[m, k] = round(min(7*|x|, 7)) as fp8 (exact small ints)
    # written to a DRAM scratch, viewed as u16 (adjacent k pairs) and
    # xbar-transposed so stage[p, m] holds k-pair (2*(kc*128+p), +1),
    # then de-interleaved into the resident aT (DR pairing even/odd k).
    # Emission is interleaved with phase B so neither engine's in-order
    # stream serializes phase B behind all of phase A.
    aq_dram = dram_pool.tile([M, IN_CH], FP8, name="aq_dram")
    aq_u16 = aq_dram[:].bitcast(mybir.dt.uint16)
    KCP = KC // 2  # kc pair-tiles per W chunk

    # Prefetch all x tiles first (split across both HWDGE rings) so the
    # ring FIFOs never stall behind the quant chain.
    xtiles = {}
    for i, (kh, mt) in enumerate(
        [(kh, mt) for kh in range(IN_CH // KH) for mt in range(MT)]
    ):
        xs = xs_pool.tile([P, KH], F32, tag="xs", name=f"xs{kh}_{mt}")
        eng = nc.sync if i % 2 == 0 else nc.scalar
        eng.dma_start(xs[:], xr[mt][:, kh * KH : (kh + 1) * KH])
        xtiles[(kh, mt)] = xs

    def emit_quant(kh):
        for mt in range(MT):
            xs = xtiles[(kh, mt)]
            # fl(7|x|) on ACT, clip+magic-add on DVE, -2^23 + fp8 cast on
            # ACT. Bit-exact vs the reference's round(min(|x|,1)*7).
            nc.scalar.activation(xs[:], xs[:], AF.Abs, scale=7.0)
            nc.vector.tensor_scalar(xs[:], xs[:], 7.0, MAGIC, ALU.min, ALU.add)
            qb = qb_pool.tile([P, KH], FP8, tag="qb", name=f"qb{kh}_{mt}")
            nc.scalar.activation(qb[:], xs[:], AF.Copy, bias=-MAGIC)
            nc.gpsimd.dma_start(
                aq_dram[mt * P : (mt + 1) * P, kh * KH : (kh + 1) * KH], qb[:]
            )

    def emit_transposes(kh):
        for kch in range(KH // 256):
            kc = (kh * KH) // 256 + kch
            stg = stg_pool.tile([P, M], mybir.dt.uint16, tag="stg", name=f"stg{kc}")
            nc.scalar.dma_start_transpose(
                out=stg[:], in_=aq_u16[:, kc * P : (kc + 1) * P]
            )
            stg8 = stg[:].bitcast(FP8).rearrange("p (m two) -> p m two", two=2)
            if use_dr:
                nc.vector.tensor_copy(aT[kc][:, 0, :], stg8[:, :, 0])
                nc.vector.tensor_copy(aT[kc][:, 1, :], stg8[:, :, 1])
            else:
                nc.vector.tensor_copy(aT[2 * kc][:], stg8[:, :, 0])
                nc.vector.tensor_copy(aT[2 * kc + 1][:], stg8[:, :, 1])

    def alloc_psums(nb):
        return [
            psum_pool.tile([P, NBS], F32, tag="ps", name=f"ps{nb}_{m}")
            for m in range(MT)
        ]

    def emit_w_chunk(nb, c, psums):
        wt = ws_pool.tile([P, KCP, 2, NBS], F32, tag="ws", name=f"wt{nb}_{c}")
        weng = nc.sync if (nb * NCH + c) % 2 == 0 else nc.gpsimd
        for j in range(KCP):
            weng.dma_start(
                wt[:, j],
                wr[:, c * KCP + j, :, nb * NBS : (nb + 1) * NBS],
            )
        st = ss_pool.tile([P, KCP, 2, NBS], FP8, tag="ss", name=f"st{nb}_{c}")
        # S' = (W>=0) - 0.5 in {+-0.5}; matmul result is then M'/2,
        # doubled at psum eviction and scaled by E/7 at the end.
        nc.vector.tensor_scalar(
            st[:], wt[:], 0.0, 0.5, ALU.is_ge, ALU.subtract
        )
        # |W| free-dim sum into an accW column, alternating engines. The
        # ACT variant writes |W| to a throwaway fp8 scratch (the fused
        # accumulator sums at fp32 before the output cast) so it has no
        # WAR dependency that would stall the ACT instruction stream.
        acol = accW[:, nb * NCH + c : nb * NCH + c + 1]
        if (nb * NCH + c) % 2 == 0:
            ascr = abs_pool.tile(
                [P, KCP, 2, NBS], FP8, tag="abss", name=f"ab{nb}_{c}"
            )
            nc.scalar.activation(ascr[:], wt[:], AF.Abs, accum_out=acol)
        else:
            nc.vector.tensor_reduce(
                acol,
                wt[:],
                axis=mybir.AxisListType.XYZ,
                op=ALU.add,
                apply_absolute_value=True,
            )
        for m in range(MT):
            if use_dr:
                for j in range(KCP):
                    nc.tensor.matmul(
                        psums[m][:],
                        aT[c * KCP + j][:, :, m * P : (m + 1) * P],
                        st[:, j, :, :],
                        start=(c == 0 and j == 0),
                        stop=(c == NCH - 1 and j == KCP - 1),
                        perf_mode=mybir.MatmulPerfMode.DoubleRow,
                    )
            else:
                for j in range(KCP):
                    for t in range(2):
                        nc.tensor.matmul(
                            psums[m][:],
                            aT[2 * (c * KCP + j) + t][:, m * P : (m + 1) * P],
                            st[:, j, t, :],
                            start=(c == 0 and j == 0 and t == 0),
                            stop=(c == NCH - 1 and j == KCP - 1 and t == 1),
                        )

    def emit_evicts(nb, psums):
        # psum holds M'/2 (half-integers when rowsum(a) is odd); double to
        # exact integers before the int16 store. Split across engines.
        for m in range(MT):
            dst = unscaled[m][:, nb * NBS : (nb + 1) * NBS]
            if m % 2 == 0:
                nc.vector.tensor_scalar(dst, psums[m][:], 2.0, None, ALU.mult)
            else:
                nc.scalar.activation(dst, psums[m][:], AF.Copy, scale=2.0)

    NQ = IN_CH // KH  # quant quarters
    emit_quant(0)
    emit_quant(1)
    emit_transposes(0)
    psums0 = alloc_psums(0)
    emit_w_chunk(0, 0, psums0)
    emit_w_chunk(0, 1, psums0)
    emit_quant(2)
    emit_transposes(1)
    emit_w_chunk(0, 2, psums0)
    emit_w_chunk(0, 3, psums0)
    emit_quant(3)
    emit_transposes(2)
    emit_w_chunk(0, 4, psums0)
    emit_w_chunk(0, 5, psums0)
    emit_transposes(3)
    emit_w_chunk(0, 6, psums0)
    emit_w_chunk(0, 7, psums0)
    emit_evicts(0, psums0)
    for nb in range(1, NB):
        psums = alloc_psums(nb)
        for c in range(NCH):
            emit_w_chunk(nb, c, psums)
        emit_evicts(nb, psums)

    # ---- Phase C: E = mean|W|; scale = E/7 ----
    accT = const.tile([P, 1], F32, name="accT")
    nc.vector.tensor_reduce(
        accT[:], accW[:], axis=mybir.AxisListType.X, op=ALU.add
    )
    accB = const.tile([P, 1], F32, name="accB")
    nc.gpsimd.partition_all_reduce(
        accB[:], accT[:], channels=P, reduce_op=bass_isa.ReduceOp.add
    )
    nc.vector.tensor_scalar(
        sAP[:], accB[:], 1.0 / (7.0 * IN_CH * N_UNITS), None, ALU.mult
    )

    # ---- Phase D: out = unscaled * (E/7) + b ----
    OBS = 2 * NBS  # coalesce two n-blocks per output DMA
    for m in range(MT):
        for ob in range(N_UNITS // OBS):
            sl = slice(ob * OBS, (ob + 1) * OBS)
            orow = orow_pool.tile([P, OBS], F32, tag="orow", name=f"or{m}_{ob}")
            nc.vector.tensor_scalar(
                orow[:], unscaled[m][:, sl], sAP[:], None, ALU.mult
            )
            if add_bias:
                nc.vector.tensor_tensor(
                    orow[:], orow[:], b_bc[:, sl], ALU.add
                )
            oeng = nc.sync if (m + ob) % 2 == 0 else nc.scalar
            oeng.dma_start(outr[m][:, sl], orow[:])


def build(use_dr=True, add_bias=True):
    nc = bacc.Bacc(
        "TRN2", target_bir_lowering=False, debug=False, num_devices=N_CORES
    )
    x = nc.dram_tensor("inputs", [M, IN_CH], F32, kind="ExternalInput").ap()
    w = nc.dram_tensor("W", [IN_CH, N_UNITS], F32, kind="ExternalInput").ap()
    b = nc.dram_tensor("b", [1, N_UNITS], F32, kind="ExternalInput").ap()
    out = nc.dram_tensor("out", [M, N_UNITS], F32, kind="ExternalOutput").ap()
    with tile.TileContext(nc) as tc, ExitStack() as ctx:
        _body(ctx, tc, x, w, b, out, use_dr, add_bias)
    nc.compile()
    return nc


_cached = {}


def _get_nc(add_bias):
    key = add_bias
    if key not in _cached:
        _cached[key] = build(use_dr=True, add_bias=add_bias)
    return _cached[key]


def _expected_inputs(nc):
    import concourse.mybir as mb

    names = set()
    for alloc in nc.m.functions[0].allocations:
        if isinstance(alloc, mb.MemoryLocationSet) and alloc.kind == "ExternalInput":
            names.add(alloc.memorylocations[0].name)
    return names


def run(inputs, W, b, trace=False):
    add_bias = bool(np.any(b))
    nc = _get_nc(add_bias)
    want = _expected_inputs(nc)
    b2 = np.ascontiguousarray(b.reshape(1, -1).astype(np.float32, copy=False))
    Wc = np.ascontiguousarray(W.astype(np.float32, copy=False))
    in_maps = []
    for c in range(N_CORES):
        shard = np.ascontiguousarray(inputs[c * M : (c + 1) * M])
        full = {"inputs": shard, "W": Wc, "b": b2}
        in_maps.append({k: v for k, v in full.items() if k in want})
    res = run_bass_kernel_spmd(
        nc, in_maps, core_ids=list(range(N_CORES)), trace=trace
    )
    out = np.concatenate([res.results[c]["out"] for c in range(N_CORES)], axis=0)
    return out, res


def kernel(inputs, W, b):
    out, _ = run(inputs, W, b, trace=False)
    return out


if __name__ == "__main__":
    rng = np.random.default_rng(0)
    x = rng.standard_normal((BATCH, IN_CH), dtype=np.float32)
    W = (rng.standard_normal((IN_CH, N_UNITS)) * 0.1).astype(np.float32)
    b = np.zeros(N_UNITS, dtype=np.float32)
    got = kernel(inputs=x, W=W, b=b)
    E = np.abs(W).mean(dtype=np.float64)
    a = np.round(np.minimum(np.abs(x), 1.0) * 7.0)
    want = (a.astype(np.float64) @ np.sign(W).astype(np.float64)) * (E / 7.0)
    err = np.abs(got - want).max() / np.abs(want).max()
    print("rel err vs numpy ref:", err)


# revision 47
# speedup vs baseline: 1.1058x; 1.1058x over previous
"""DoReFa dense layer (bitW=1, bitA=3) on 8 Trainium2 NeuronCores.

out = quantize_act(clip(|x|,0,1), 3b) @ (sign(W) * mean|W|) + b

Math used by the kernel (exact):
    a_int = round(min(7*|x|, 7))   in {0..7}   -> exact in bf16/fp8
    S     = sign(W)                in {-1,0,1} -> exact in fp8
    out   = (E/7) * (a_int @ S) + b,  E = mean|W| (computed on device)

The integer matmul accumulates exactly in fp32 PSUM (|sums| <= 28672 < 2^15),
so intermediate results are stored as int16 and scaled by E/7 at the end.

Sharding: data-parallel over batch (8 x 1024 rows), W replicated.
"""

import sys

sys.path.insert(0, "/opt/trn_rl_repo")

from contextlib import ExitStack

import numpy as np
from concourse import bacc, mybir, tile
from concourse import bass_isa
from concourse.bass_utils import run_bass_kernel_spmd

# Problem dims (hardcoded per contract)
BATCH, IN_CH, N_UNITS = 8192, 4096, 4096
N_CORES = 8
P = 128

M = BATCH // N_CORES  # 1024 rows per core
KO = IN_CH // P  # 32 k-subtiles of 128
MT = M // P  # 8 m-subtiles of 128
NBS = 512  # n-block width
NB = N_UNITS // NBS  # 8 n-blocks
KC = 4  # k-subtiles per W dma chunk
NCH = KO // KC  # 8 chunks per n-block
KH = 1024  # k columns per activation quant chunk

MAGIC = float(2**23)

F32 = mybir.dt.float32
BF16 = mybir.dt.bfloat16
FP8 = mybir.dt.float8e4
I16 = mybir.dt.int16
AF = mybir.ActivationFunctionType
ALU = mybir.AluOpType


def _body(ctx, tc, x, w, b, out, use_dr, add_bias):
    nc = tc.nc

    xr = x.rearrange("(mt p) k -> mt p k", p=P)
    # row = kc*256 + 2p + t: partition p holds the adjacent row pair
    # (2p, 2p+1) of each 256-row group kc -- matches the aT u16 pairing.
    wr = w.rearrange("(kc p two) n -> p kc two n", p=P, two=2)
    outr = out.rearrange("(mt p) n -> mt p n", p=P)

    const = ctx.enter_context(tc.tile_pool(name="const", bufs=1))
    xs_pool = ctx.enter_context(tc.tile_pool(name="xs", bufs=3))
    qb_pool = ctx.enter_context(tc.tile_pool(name="qb", bufs=2))
    stg_pool = ctx.enter_context(tc.tile_pool(name="stg", bufs=4))
    ws_pool = ctx.enter_context(tc.tile_pool(name="ws", bufs=4))
    ss_pool = ctx.enter_context(tc.tile_pool(name="ss", bufs=9))
    abs_pool = ctx.enter_context(tc.tile_pool(name="abss", bufs=2))
    orow_pool = ctx.enter_context(tc.tile_pool(name="orow", bufs=3))
    psum_pool = ctx.enter_context(tc.tile_pool(name="psum", bufs=8, space="PSUM"))
    dram_pool = ctx.enter_context(tc.tile_pool(name="dram", bufs=1, space="DRAM"))

    # Resident tensors (all fp8 activations: 32KB/partition)
    if use_dr:
        # ko-pair tiles for DoubleRow: [p, 2, M] fp8
        aT = [const.tile([P, 2, M], FP8, name=f"aT{i}") for i in range(KO // 2)]
    else:
        aT = [const.tile([P, M], FP8, name=f"aT{i}") for i in range(KO)]
    unscaled = [const.tile([P, N_UNITS], I16, name=f"uns{m}") for m in range(MT)]
    accW = const.tile([P, NB * NCH], F32, name="accW")
    sAP = const.tile([P, 1], F32, name="sAP")

    if add_bias:
        b_bc = const.tile([P, N_UNITS], F32, name="b_bc")
        nc.scalar.dma_start(b_bc[0:1, :], b[:])
        nc.gpsimd.partition_broadcast(b_bc[:], b_bc[0:1, :], channels=P)

    # ---- Phase A + B interleaved ----
    # Phase A: a_q[m, k] = round(min(7*|x|, 7)) as fp8 (exact small ints)
    # written to a DRAM scratch, viewed as u16 (adjacent k pairs) and
    # xbar-transposed so stage[p, m] holds k-pair (2*(kc*128+p), +1),
    # then de-interleaved into the resident aT (DR pairing even/odd k).
    # Emission is interleaved with phase B so neither engine's in-order
    # stream serializes phase B behind all of phase A.
    aq_dram = dram_pool.tile([M, IN_CH], FP8, name="aq_dram")
    aq_u16 = aq_dram[:].bitcast(mybir.dt.uint16)
    KCP = KC // 2  # kc pair-tiles per W chunk

    # Prefetch all x tiles first (split across both HWDGE rings) so the
    # ring FIFOs never stall behind the quant chain.
    xtiles = {}
    for i, (kh, mt) in enumerate(
        [(kh, mt) for kh in range(IN_CH // KH) for mt in range(MT)]
    ):
        xs = xs_pool.tile([P, KH], F32, tag="xs", name=f"xs{kh}_{mt}")
        eng = nc.sync if i % 2 == 0 else nc.scalar
        eng.dma_start(xs[:], xr[mt][:, kh * KH : (kh + 1) * KH])
        xtiles[(kh, mt)] = xs

    def emit_quant(kh):
        for mt in range(MT):
            xs = xtiles[(kh, mt)]
            # |x| via sign-bit clear, then fl(7|x|) clipped to 7, then the
            # +-2^23 magic rounds to nearest-even inside the fp32 ALU
            # chain. Bit-exact vs the reference's round(min(|x|,1)*7).
            xu = xs[:].bitcast(mybir.dt.uint32)
            nc.vector.tensor_scalar(xu, xu, 0x7FFFFFFF, None, ALU.bitwise_and)
            nc.vector.tensor_scalar(xs[:], xs[:], 7.0, 7.0, ALU.mult, ALU.min)
            qb = qb_pool.tile([P, KH], FP8, tag="qb", name=f"qb{kh}_{mt}")
            nc.vector.tensor_scalar(
                qb[:], xs[:], MAGIC, MAGIC, ALU.add, ALU.subtract
            )
            nc.gpsimd.dma_start(
                aq_dram[mt * P : (mt + 1) * P, kh * KH : (kh + 1) * KH], qb[:]
            )

    def emit_transposes(kh):
        for kch in range(KH // 256):
            kc = (kh * KH) // 256 + kch
            stg = stg_pool.tile([P, M], mybir.dt.uint16, tag="stg", name=f"stg{kc}")
            nc.sync.dma_start_transpose(
                out=stg[:], in_=aq_u16[:, kc * P : (kc + 1) * P]
            )
            stg8 = stg[:].bitcast(FP8).rearrange("p (m two) -> p m two", two=2)
            if use_dr:
                nc.scalar.copy(aT[kc][:, 0, :], stg8[:, :, 0])
                nc.scalar.copy(aT[kc][:, 1, :], stg8[:, :, 1])
            else:
                nc.scalar.copy(aT[2 * kc][:], stg8[:, :, 0])
                nc.scalar.copy(aT[2 * kc + 1][:], stg8[:, :, 1])

    def alloc_psums(nb):
        return [
            psum_pool.tile([P, NBS], F32, tag="ps", name=f"ps{nb}_{m}")
            for m in range(MT)
        ]

    def emit_w_chunk(nb, c, psums):
        wt = ws_pool.tile([P, KCP, 2, NBS], F32, tag="ws", name=f"wt{nb}_{c}")
        weng = nc.scalar if (nb * NCH + c) % 2 == 0 else nc.sync
        for j in range(KCP):
            weng.dma_start(
                wt[:, j],
                wr[:, c * KCP + j, :, nb * NBS : (nb + 1) * NBS],
            )
        st = ss_pool.tile([P, KCP, 2, NBS], FP8, tag="ss", name=f"st{nb}_{c}")
        # S' = (W>=0) - 0.5 in {+-0.5}; matmul result is then M'/2,
        # doubled at psum eviction and scaled by E/7 at the end.
        nc.vector.tensor_scalar(
            st[:], wt[:], 0.0, 0.5, ALU.is_ge, ALU.subtract
        )
        # |W| free-dim sum into an accW column, alternating engines. The
        # ACT variant writes |W| to a throwaway fp8 scratch (the fused
        # accumulator sums at fp32 before the output cast) so it has no
        # WAR dependency that would stall the ACT instruction stream.
        acol = accW[:, nb * NCH + c : nb * NCH + c + 1]
        if nb < 2 or (nb * NCH + c) % 2 == 0:
            ascr = abs_pool.tile(
                [P, KCP, 2, NBS], FP8, tag="abss", name=f"ab{nb}_{c}"
            )
            nc.scalar.activation(ascr[:], wt[:], AF.Abs, accum_out=acol)
        else:
            nc.vector.tensor_reduce(
                acol,
                wt[:],
                axis=mybir.AxisListType.XYZ,
                op=ALU.add,
                apply_absolute_value=True,
            )
        for m in range(MT):
            if use_dr:
                for j in range(KCP):
                    nc.tensor.matmul(
                        psums[m][:],
                        aT[c * KCP + j][:, :, m * P : (m + 1) * P],
                        st[:, j, :, :],
                        start=(c == 0 and j == 0),
                        stop=(c == NCH - 1 and j == KCP - 1),
                        perf_mode=mybir.MatmulPerfMode.DoubleRow,
                    )
            else:
                for j in range(KCP):
                    for t in range(2):
                        nc.tensor.matmul(
                            psums[m][:],
                            aT[2 * (c * KCP + j) + t][:, m * P : (m + 1) * P],
                            st[:, j, t, :],
                            start=(c == 0 and j == 0 and t == 0),
                            stop=(c == NCH - 1 and j == KCP - 1 and t == 1),
                        )

    def emit_evicts(nb, psums):
        # psum holds M'/2 (half-integers when rowsum(a) is odd); double to
        # exact integers before the int16 store. Split across engines.
        for m in range(MT):
            dst = unscaled[m][:, nb * NBS : (nb + 1) * NBS]
            if m % 2 == 0:
                nc.vector.tensor_scalar(dst, psums[m][:], 2.0, None, ALU.mult)
            else:
                nc.scalar.activation(dst, psums[m][:], AF.Copy, scale=2.0)

    NQ = IN_CH // KH  # quant quarters
    emit_quant(0)
    emit_quant(1)
    emit_transposes(0)
    psums0 = alloc_psums(0)
    emit_w_chunk(0, 0, psums0)
    emit_w_chunk(0, 1, psums0)
    emit_quant(2)
    emit_transposes(1)
    emit_w_chunk(0, 2, psums0)
    emit_w_chunk(0, 3, psums0)
    emit_quant(3)
    emit_transposes(2)
    emit_w_chunk(0, 4, psums0)
    emit_w_chunk(0, 5, psums0)
    emit_transposes(3)
    emit_w_chunk(0, 6, psums0)
    emit_w_chunk(0, 7, psums0)
    emit_evicts(0, psums0)
    for nb in range(1, NB):
        psums = alloc_psums(nb)
        for c in range(NCH):
            emit_w_chunk(nb, c, psums)
        emit_evicts(nb, psums)

    # ---- Phase C: E = mean|W|; scale = E/7 ----
    accT = const.tile([P, 1], F32, name="accT")
    nc.vector.tensor_reduce(
        accT[:], accW[:], axis=mybir.AxisListType.X, op=ALU.add
    )
    accB = const.tile([P, 1], F32, name="accB")
    nc.gpsimd.partition_all_reduce(
        accB[:], accT[:], channels=P, reduce_op=bass_isa.ReduceOp.add
    )
    nc.vector.tensor_scalar(
        sAP[:], accB[:], 1.0 / (7.0 * IN_CH * N_UNITS), None, ALU.mult
    )

    # ---- Phase D: out = unscaled * (E/7) + b ----
    OBS = 2 * NBS  # coalesce two n-blocks per output DMA
    for m in range(MT):
        for ob in range(N_UNITS // OBS):
            sl = slice(ob * OBS, (ob + 1) * OBS)
            orow = orow_pool.tile([P, OBS], F32, tag="orow", name=f"or{m}_{ob}")
            nc.vector.tensor_scalar(
                orow[:], unscaled[m][:, sl], sAP[:], None, ALU.mult
            )
            if add_bias:
                nc.vector.tensor_tensor(
                    orow[:], orow[:], b_bc[:, sl], ALU.add
                )
            oeng = nc.sync if (m + ob) % 2 == 0 else nc.scalar
            oeng.dma_start(outr[m][:, sl], orow[:])


def build(use_dr=True, add_bias=True):
    nc = bacc.Bacc(
        "TRN2", target_bir_lowering=False, debug=False, num_devices=N_CORES
    )
    x = nc.dram_tensor("inputs", [M, IN_CH], F32, kind="ExternalInput").ap()
    w = nc.dram_tensor("W", [IN_CH, N_UNITS], F32, kind="ExternalInput").ap()
    b = nc.dram_tensor("b", [1, N_UNITS], F32, kind="ExternalInput").ap()
    out = nc.dram_tensor("out", [M, N_UNITS], F32, kind="ExternalOutput").ap()
    with tile.TileContext(nc) as tc, ExitStack() as ctx:
        _body(ctx, tc, x, w, b, out, use_dr, add_bias)
    nc.compile()
    return nc


_cached = {}


def _get_nc(add_bias):
    key = add_bias
    if key not in _cached:
        _cached[key] = build(use_dr=True, add_bias=add_bias)
    return _cached[key]


def _expected_inputs(nc):
    import concourse.mybir as mb

    names = set()
    for alloc in nc.m.functions[0].allocations:
        if isinstance(alloc, mb.MemoryLocationSet) and alloc.kind == "ExternalInput":
            names.add(alloc.memorylocations[0].name)
    return names


def run(inputs, W, b, trace=False):
    add_bias = bool(np.any(b))
    nc = _get_nc(add_bias)
    want = _expected_inputs(nc)
    b2 = np.ascontiguousarray(b.reshape(1, -1).astype(np.float32, copy=False))
    Wc = np.ascontiguousarray(W.astype(np.float32, copy=False))
    in_maps = []
    for c in range(N_CORES):
        shard = np.ascontiguousarray(inputs[c * M : (c + 1) * M])
        full = {"inputs": shard, "W": Wc, "b": b2}
        in_maps.append({k: v for k, v in full.items() if k in want})
    res = run_bass_kernel_spmd(
        nc, in_maps, core_ids=list(range(N_CORES)), trace=trace
    )
    out = np.concatenate([res.results[c]["out"] for c in range(N_CORES)], axis=0)
    return out, res


def kernel(inputs, W, b):
    out, _ = run(inputs, W, b, trace=False)
    return out


if __name__ == "__main__":
    rng = np.random.default_rng(0)
    x = rng.standard_normal((BATCH, IN_CH), dtype=np.float32)
    W = (rng.standard_normal((IN_CH, N_UNITS)) * 0.1).astype(np.float32)
    b = np.zeros(N_UNITS, dtype=np.float32)
    got = kernel(inputs=x, W=W, b=b)
    E = np.abs(W).mean(dtype=np.float64)
    a = np.round(np.minimum(np.abs(x), 1.0) * 7.0)
    want = (a.astype(np.float64) @ np.sign(W).astype(np.float64)) * (E / 7.0)
    err = np.abs(got - want).max() / np.abs(want).max()
    print("rel err vs numpy ref:", err)
